# revision 8
# baseline (speedup 1.0000x reference)
"""Trainium2 Bass kernel for a hybrid classical/quantum head.

Math: the reference is  out = Q(tanh(X @ Wpre.T + bpre) * pi/2) @ Wpost.T + bpost
where Q() simulates a 10-qubit circuit: H on all wires, per-sample RY(theta_w),
then 6 layers of (CNOT chain + shared RY(qw)), returning PauliZ expvals.

Restructuring used here:
  * After H + per-sample RY, the state is a PRODUCT state:
      s2[j] = prod_w v_w(bit_w(j)),  v_w(0)=cos(phi_w), v_w(1)=sin(phi_w),
      phi_w = theta_w/2 + pi/4,  theta_w = tanh(pre)*pi/2.
  * Everything after is a fixed linear operator A (1024x1024) that depends only
    on q_params -> built host-side in fp64 (tiny), shipped as fp16.
  * z_w = sum_j sign_w(j) * (A s2)_j^2, and the post-linear folds in:
      out[s, c] = sum_j d[c, j] * y[s, j]^2 + bpost[c],  d = Wpost @ Sgn.

Per-core device pipeline (data-parallel over batch, 1024 samples/core):
  prenet matmul (f32, PE) -> tanh/sin (ACT) -> kron-factor build (GPSIMD)
  -> broadcast-AP PE transposes -> expand to S2^T fp16 (DVE)
  -> Y^T = A @ S2^T (fp16 PE matmul, f32 accum) -> square (ACT)
  -> d-contraction (fp16 PE matmul) -> +bias -> out^T (2, 1024) f32.
"""

import numpy as np

N_QUBITS = 10
Q_DEPTH = 6
MAX_LAYERS = 15
DIM = 2**N_QUBITS
N_CORES = 8
B_FULL = 8192
F_IN = 512
N_CLS = 2
BC = B_FULL // N_CORES  # 1024 samples per core
P = 128

_CACHE = {}


def _build_A(q_params):
    """Fixed circuit operator after the per-sample RY layer, fp64 on host."""
    qp = np.asarray(q_params, np.float64)
    qw = qp.reshape(MAX_LAYERS, N_QUBITS)
    N = N_QUBITS

    def apply_1q(M, U, w):
        a, b = 2**w, 2 ** (N - 1 - w)
        M = M.reshape(a, 2, b, DIM)
        M = np.einsum('ij,ajbk->aibk', U, M)
        return M.reshape(DIM, DIM)

    def apply_cnot(M, c, t):
        M = M.reshape(2**c, 2, 2 ** (t - c - 1), 2, 2 ** (N - 1 - t), DIM)
        M = np.stack([M[:, 0], np.flip(M[:, 1], axis=2)], axis=1)
        return M.reshape(DIM, DIM)

    def ry(th):
        c, s = np.cos(th / 2), np.sin(th / 2)
        return np.array([[c, -s], [s, c]])

    A = np.eye(DIM)
    for k in range(Q_DEPTH):
        for i in range(0, N - 1, 2):
            A = apply_cnot(A, i, i + 1)
        for i in range(1, N - 1, 2):
            A = apply_cnot(A, i, i + 1)
        for w in range(N):
            A = apply_1q(A, ry(qw[k + 1, w]), w)
    return A


def _build_bass():
    import concourse.mybir as mybir
    from concourse import bacc
    from concourse.masks import make_identity
    from concourse.tile import TileContext

    dt = mybir.dt
    AF = mybir.ActivationFunctionType
    ALU = mybir.AluOpType
    PI = float(np.pi)

    nc = bacc.Bacc()
    xT = nc.dram_tensor("xT", [F_IN, BC], dt.float32, kind="ExternalInput")
    wpre = nc.dram_tensor("wpre", [4, P, N_QUBITS], dt.float32, kind="ExternalInput")
    bpre = nc.dram_tensor("bpre", [P, N_QUBITS], dt.float32, kind="ExternalInput")
    aT = nc.dram_tensor("aT", [DIM, DIM], dt.float16, kind="ExternalInput")
    dT = nc.dram_tensor("dT", [DIM, N_CLS], dt.float16, kind="ExternalInput")
    bpost = nc.dram_tensor("bpost", [N_CLS, 1], dt.float32, kind="ExternalInput")
    outT = nc.dram_tensor("outT", [N_CLS, BC], dt.float32, kind="ExternalOutput")

    NSUB = BC // P           # 8 sample sub-tiles
    NKT = DIM // P           # 8 k (amplitude) tiles
    NCH = 2                  # two 512-wide sample chunks for the big matmuls
    CW = BC // NCH           # 512

    with TileContext(nc) as tc:
        with (
            tc.tile_pool(name="const", bufs=1) as cpool,
            tc.tile_pool(name="small", bufs=3) as spool,
            tc.tile_pool(name="ps_pre", bufs=2, space="PSUM") as ps_pre,
            tc.tile_pool(name="ps_tr", bufs=3, space="PSUM") as ps_tr,
            tc.tile_pool(name="ps_y", bufs=2, space="PSUM") as ps_y,
            tc.tile_pool(name="ps_o", bufs=1, space="PSUM") as ps_o,
        ):
            ident = cpool.tile([P, P], dt.float32)
            make_identity(nc, ident)
            bias14 = cpool.tile([P, 1], dt.float32)
            nc.gpsimd.memset(bias14, PI / 4.0)
            bias34 = cpool.tile([P, 1], dt.float32)
            nc.gpsimd.memset(bias34, 3.0 * PI / 4.0)

            wpre_sb = cpool.tile([P, 4, N_QUBITS], dt.float32)
            nc.sync.dma_start(wpre_sb, wpre[:].rearrange("a p q -> p a q"))
            bpre_sb = cpool.tile([P, N_QUBITS], dt.float32)
            nc.sync.dma_start(bpre_sb, bpre[:])
            dT_sb = cpool.tile([P, NKT, N_CLS], dt.float16)
            nc.sync.dma_start(dT_sb, dT[:].rearrange("(t p) c -> p t c", p=P))
            bpost_sb = cpool.tile([N_CLS, 1], dt.float32)
            nc.sync.dma_start(bpost_sb, bpost[:])
            xT_sb = cpool.tile([P, 4, BC], dt.float32)
            nc.sync.dma_start(xT_sb, xT[:].rearrange("(a p) s -> p a s", p=P))
            aT_sb = cpool.tile([P, NKT, DIM], dt.float16)
            nc.sync.dma_start(aT_sb, aT[:].rearrange("(t p) j -> p t j", p=P))

            # persistent staging for prep phase
            q_all = cpool.tile([P, NSUB, N_QUBITS], dt.float32)
            v0_all = cpool.tile([P, NSUB, N_QUBITS], dt.float32)
            v1_all = cpool.tile([P, NSUB, N_QUBITS], dt.float32)
            shi_all = cpool.tile([P, NSUB, 32], dt.float32)
            slo_all = cpool.tile([P, NSUB, 32], dt.float32)
            sloT_all = cpool.tile([P, NSUB, P], dt.float32)
            s2T = [cpool.tile([P, NKT, CW], dt.float16, name=f"s2T{c}") for c in range(NCH)]
            p_all = [cpool.tile([P, NKT, CW], dt.float16, name=f"p_all{c}") for c in range(NCH)]
            outT_sb = cpool.tile([N_CLS, BC], dt.float32)

            # ---- stage 1: prenet matmuls + bias (all subs) ----
            preb_list = []
            for sub in range(NSUB):
                ssl = slice(sub * P, (sub + 1) * P)
                pre_ps = ps_pre.tile([P, N_QUBITS], dt.float32)
                for ft in range(4):
                    nc.tensor.matmul(
                        pre_ps, xT_sb[:, ft, ssl], wpre_sb[:, ft, :],
                        start=(ft == 0), stop=(ft == 3),
                    )
                preb = spool.tile([P, N_QUBITS], dt.float32, name=f"preb{sub}")
                # preb = pre + b_pre
                nc.vector.scalar_tensor_tensor(
                    preb, pre_ps, 1.0, bpre_sb, ALU.mult, ALU.add
                )
                preb_list.append(preb)

            # ---- stage 2: activations (ACT) ----
            for sub in range(NSUB):
                nc.scalar.activation(q_all[:, sub, :], preb_list[sub], AF.Tanh)
            for sub in range(NSUB):
                # theta = q * pi/2 ; phi = theta/2 + pi/4 ; v0 = cos phi, v1 = sin phi
                nc.scalar.activation(
                    v0_all[:, sub, :], q_all[:, sub, :], AF.Sin,
                    bias=bias34[:, 0:1], scale=PI / 4.0,
                )
                nc.scalar.activation(
                    v1_all[:, sub, :], q_all[:, sub, :], AF.Sin,
                    bias=bias14[:, 0:1], scale=PI / 4.0,
                )

            # ---- stage 3: kron-factor builds (GPSIMD; tiny columns) ----
            def build_half(dst, sub, wires):
                # dst[:, sub, :]: 32 columns = product over 5 wires, first wire
                # in `wires` ends up the most-significant bit.
                v0 = v0_all[:, sub, :]
                v1 = v1_all[:, sub, :]
                t2 = spool.tile([P, 2], dt.float32, name=f"t2_{sub}")
                t4 = spool.tile([P, 4], dt.float32, name=f"t4_{sub}")
                t8 = spool.tile([P, 8], dt.float32, name=f"t8_{sub}")
                t16 = spool.tile([P, 16], dt.float32, name=f"t16_{sub}")
                w4, w3, w2, w1, w0 = wires[4], wires[3], wires[2], wires[1], wires[0]
                g = nc.gpsimd
                g.tensor_copy(t2[:, 0:1], v0[:, w4:w4 + 1])
                g.tensor_copy(t2[:, 1:2], v1[:, w4:w4 + 1])
                g.tensor_scalar_mul(t4[:, 0:2], t2, v0[:, w3:w3 + 1])
                g.tensor_scalar_mul(t4[:, 2:4], t2, v1[:, w3:w3 + 1])
                g.tensor_scalar_mul(t8[:, 0:4], t4, v0[:, w2:w2 + 1])
                g.tensor_scalar_mul(t8[:, 4:8], t4, v1[:, w2:w2 + 1])
                g.tensor_scalar_mul(t16[:, 0:8], t8, v0[:, w1:w1 + 1])
                g.tensor_scalar_mul(t16[:, 8:16], t8, v1[:, w1:w1 + 1])
                g.tensor_scalar_mul(dst[:, sub, 0:16], t16, v0[:, w0:w0 + 1])
                g.tensor_scalar_mul(dst[:, sub, 16:32], t16, v1[:, w0:w0 + 1])

            for sub in range(NSUB):
                build_half(shi_all, sub, [0, 1, 2, 3, 4])
                build_half(slo_all, sub, [5, 6, 7, 8, 9])

            # ---- stage 4: expanded transposes (PE) + S2^T expand (DVE) ----
            for sub in range(NSUB):
                ch, csub = sub // 4, sub % 4
                csl = slice(csub * P, (csub + 1) * P)
                # materialize expanded operands (matmul weight APs allow only
                # one free dim, so broadcast APs can't feed transpose directly)
                slo_exp = spool.tile([P, 4, 32], dt.float32, name=f"sloe{sub}", tag="sloe")
                nc.gpsimd.tensor_copy(
                    slo_exp, slo_all[:, sub, None, :].broadcast_to((P, 4, 32))
                )
                slo_ps = ps_tr.tile([P, P], dt.float32, name=f"slo_ps{sub}", tag="tr")
                # columns: [slo_0..slo_31] repeated 4x -> partition r = jlo pattern
                nc.tensor.transpose(slo_ps, slo_exp, ident)
                # keep one SBUF copy (DVE can't read two PSUM operands)
                nc.scalar.copy(sloT_all[:, sub, :], slo_ps)
                for jt in range(NKT):
                    shi_exp = spool.tile(
                        [P, 4, 32], dt.float32, name=f"shie{sub}_{jt}", tag="shie"
                    )
                    nc.gpsimd.tensor_copy(
                        shi_exp,
                        shi_all[:, sub, 4 * jt:4 * jt + 4, None].broadcast_to((P, 4, 32)),
                    )
                    shi_ps = ps_tr.tile([P, P], dt.float32, name=f"shi_ps{sub}_{jt}", tag="tr")
                    nc.tensor.transpose(shi_ps, shi_exp, ident)
                    nc.vector.tensor_mul(
                        s2T[ch][:, jt, csl], shi_ps, sloT_all[:, sub, :]
                    )

            # ---- stage 5: main matmul Y^T = A @ S2^T + square + d-contraction ----
            for ch in range(NCH):
                for jt in range(NKT):
                    jsl = slice(jt * P, (jt + 1) * P)
                    y_ps = ps_y.tile([P, CW], dt.float32, name=f"y_ps{ch}_{jt}", tag="y")
                    for kt in range(NKT):
                        nc.tensor.matmul(
                            y_ps, aT_sb[:, kt, jsl], s2T[ch][:, kt, :],
                            start=(kt == 0), stop=(kt == NKT - 1),
                        )
                    nc.scalar.activation(p_all[ch][:, jt, :], y_ps, AF.Square)
                out_ps = ps_o.tile([N_CLS, CW], dt.float32, name=f"out_ps{ch}", tag="o")
                for jt in range(NKT):
                    nc.tensor.matmul(
                        out_ps, dT_sb[:, jt, :], p_all[ch][:, jt, :],
                        start=(jt == 0), stop=(jt == NKT - 1),
                    )
                nc.scalar.activation(
                    outT_sb[:, ch * CW:(ch + 1) * CW], out_ps, AF.Identity,
                    bias=bpost_sb[:, 0:1],
                )

            nc.sync.dma_start(outT[:], outT_sb)

    nc.finalize()
    return nc


def _get_nc():
    if "nc" not in _CACHE:
        _CACHE["nc"] = _build_bass()
    return _CACHE["nc"]


def _prepare_in_maps(input_features, W_pre, b_pre, q_params, W_post, b_post):
    X = np.asarray(input_features, np.float32)
    A = _build_A(q_params)
    AT16 = np.ascontiguousarray(A.T).astype(np.float16)

    j = np.arange(DIM)
    sgn = np.stack(
        [1.0 - 2.0 * ((j >> (N_QUBITS - 1 - w)) & 1) for w in range(N_QUBITS)]
    )  # (10, 1024)
    d = np.asarray(W_post, np.float64) @ sgn  # (2, 1024)
    dT16 = np.ascontiguousarray(d.T).astype(np.float16)

    wpre_pack = np.ascontiguousarray(
        np.asarray(W_pre, np.float32).T.reshape(4, P, N_QUBITS)
    )
    bpre_rep = np.ascontiguousarray(
        np.broadcast_to(np.asarray(b_pre, np.float32), (P, N_QUBITS))
    )
    bpost_col = np.asarray(b_post, np.float32).reshape(N_CLS, 1)

    XT = np.asarray(X, np.float32).T  # (512, 8192)
    in_maps = []
    for c in range(N_CORES):
        in_maps.append({
            "xT": np.ascontiguousarray(XT[:, c * BC:(c + 1) * BC]),
            "wpre": wpre_pack,
            "bpre": bpre_rep,
            "aT": AT16,
            "dT": dT16,
            "bpost": bpost_col,
        })
    return in_maps


def run(inputs, trace=False):
    """Run on 8 cores; returns (output (8192, 2) f32, BassKernelResults)."""
    from concourse.bass_utils import run_bass_kernel_spmd

    nc = _get_nc()
    in_maps = _prepare_in_maps(**inputs)
    res = run_bass_kernel_spmd(
        nc, in_maps, core_ids=list(range(N_CORES)), trace=trace
    )
    out = np.empty((B_FULL, N_CLS), np.float32)
    for c in range(N_CORES):
        out[c * BC:(c + 1) * BC, :] = res.results[c]["outT"].T
    return out, res


def kernel(input_features, W_pre, b_pre, q_params, W_post, b_post):
    out, _ = run(dict(
        input_features=input_features, W_pre=W_pre, b_pre=b_pre,
        q_params=q_params, W_post=W_post, b_post=b_post,
    ))
    return out
